# revision 27
# baseline (speedup 1.0000x reference)
"""Trainium2 Bass kernel for: Conv3d(3,24,k=3,VALID) -> min over depth -> softmax over channels.

Input  x: [16,3,32,128,128] f32, conv_weight [24,3,3,3,3], conv_bias [24].
Output: [16,24,1,126,126] f32.

Strategy (per core; batch-sharded 2 samples/core over 8 cores):
 - x in SBUF as 4 "pages" of 4 h-blocks; each 32-partition block holds
   rows r = ci*10 + h_local (10 h-rows x 3 channels, 2 pad rows) with the
   free dim = (n, d, w).
 - Conv via 16-way 32x32 TensorE tiling: tile (i,j) = (h-block i, co-chunk j),
   M = 24 = (hoff4 x co6) per ho-quad rho, K covers (kh,ci) via Toeplitz
   weights, (kd,kw) = 9 PSUM-accumulation passes with shifted rhs APs.
   dtype float32r (fp32 data on the PE fast path; ~1e-3 on HW).
 - Weights NEGATED on host so min-over-depth becomes a free-dim reduce max.
 - DVE tensor_reduce(max) folds depth 30->1 straight out of PSUM.
 - ACT computes exp(bias - m) (= exp(min(y)+bias)); PE transposes exp tiles;
   ACT Copy+accum_out builds softmax denominators; ACT Copy+scale divides.
"""
import sys

sys.path.insert(0, "/opt/trn_rl_repo")

import numpy as np

# Problem constants
N_TOT, CI, D, H, W = 16, 3, 32, 128, 128
CO = 24
DO, HO, WO = 30, 126, 126
NCORES = 8
NPC = N_TOT // NCORES  # samples per core = 2

# dtype for matmul operands (PSUM accumulation is always fp32)
MM_DT = "float32r"

_cache = {}


def _np_mmdt():
    if MM_DT == "float32" or MM_DT == "float32r":
        return np.float32
    if MM_DT == "float16":
        return np.float16
    import ml_dtypes
    return ml_dtypes.bfloat16


def _build_program():
    import concourse.bass as bass
    import concourse.mybir as mybir
    from concourse import bacc, tile

    dt = mybir.dt
    mdt = getattr(dt, MM_DT)
    f32 = dt.float32
    AX = mybir.AxisListType
    ALU = mybir.AluOpType
    ACT_F = mybir.ActivationFunctionType

    nc = bacc.Bacc("TRN2", target_bir_lowering=False, debug=False)

    # host-pre-blocked x: [page, partition, n*34*130] with partition =
    # 32*i + 10*ci + h_local, (d,w) zero-padded to (34,130) for bank-fill
    xs = nc.dram_tensor("xs", [4, 128, NPC * 34 * 130], mdt,
                        kind="ExternalInput")
    wt = nc.dram_tensor("wt", [128, 72 * 32], mdt, kind="ExternalInput")
    bias = nc.dram_tensor("bias", [128, 1], f32, kind="ExternalInput")
    ident = nc.dram_tensor("ident", [128, 128], f32, kind="ExternalInput")
    # [n, ho, wo, co] so output DMA descriptors are 96B-contiguous runs
    out = nc.dram_tensor("out", [NPC, HO, WO, CO], f32, kind="ExternalOutput")

    with tile.TileContext(nc) as tc:
        with (
            tc.tile_pool(name="const", bufs=1) as constp,
            tc.tile_pool(name="xpage", bufs=2) as xpagep,
            tc.tile_pool(name="expp", bufs=6) as expp,
            tc.tile_pool(name="soft", bufs=4) as softp,
            tc.tile_pool(name="ps", bufs=2, space="PSUM") as psp,
        ):
            wt_t = constp.tile([128, 72 * 32], mdt)
            nc.sync.dma_start(wt_t[:], wt[:])
            bias_t = constp.tile([128, 1], f32)
            nc.sync.dma_start(bias_t[:], bias[:])
            ident_t = constp.tile([128, 128], f32)
            nc.sync.dma_start(ident_t[:], ident[:])

            DP, WP = 34, 130  # padded (d, w) extents per partition
            FREE = NPC * DP * WP  # free elems per partition of a page

            pending = []  # queue of epilogue-2 closures (lagged)
            gctr = [0]  # rotates the transpose's borrowed PSUM tag

            for page in range(4):
                page_t = xpagep.tile([128, FREE], mdt)
                pv = page_t[:].rearrange("p (n d w) -> p n d w", n=NPC, d=DP, w=WP)
                nc.sync.dma_start(page_t[:], xs[page])

                for rho in range(2):
                    for n in range(NPC):
                        exp_ts = [expp.tile([128, 128], f32, tag=f"exp{i}",
                                            name=f"exp_{i}")
                                  for i in range(4)]
                        for wc in range(8):
                            convs, cvs = [], []
                            for i in range(4):
                                conv_t = psp.tile([128, 512], f32,
                                                  tag=f"conv{i}")
                                convs.append(conv_t)
                                cvs.append(conv_t[:, 0:480].rearrange(
                                    "p (w d) -> p w d", d=30))
                            # i innermost: the 4 row-tiles stream
                            # concurrently in the 32x128-tiled PE array
                            for p9 in range(9):
                                kd, kw = p9 // 3, p9 % 3
                                m = (rho * 9) + kd * 3 + kw
                                for i in range(4):
                                    lhsT = wt_t[32 * i:32 * i + 32,
                                                128 * m:128 * m + 128]
                                    rhs = pv[32 * i:32 * i + 32, n,
                                             kd:kd + 30,
                                             wc * 16 + kw:wc * 16 + kw + 16
                                             ].rearrange("p d w -> p w d")
                                    nc.tensor.matmul(
                                        cvs[i],
                                        lhsT=lhsT,
                                        rhs=rhs,
                                        start=(p9 == 0),
                                        stop=False,
                                        tile_position=(32 * i, 0),
                                    )
                            for i in range(4):
                                # bank-fill: 32 junk cols so the PSUM zero
                                # region is fully written before the read
                                nc.tensor.matmul(
                                    convs[i][:, 480:512],
                                    lhsT=wt_t[32 * i:32 * i + 32, 0:128],
                                    rhs=pv[32 * i:32 * i + 32, n, 0, 0:32],
                                    start=False, stop=True,
                                    tile_position=(32 * i, 0),
                                )
                            for i in range(4):
                                # depth-max out of PSUM
                                nc.vector.tensor_reduce(
                                    exp_ts[i][:, wc * 16:wc * 16 + 16],
                                    cvs[i], axis=AX.X, op=ALU.max)
                        for i in range(4):
                            # exp(bias - m) in-place on the reduced tile
                            nc.scalar.activation(
                                exp_ts[i][:], exp_ts[i][:], ACT_F.Exp,
                                bias=bias_t[:, 0:1], scale=-1.0)

                        def ep2(page=page, rho=rho, n=n, exp_ts=exp_ts):
                            g = gctr[0]; gctr[0] += 1
                            tp_t = psp.tile([128, 512], f32,
                                            tag=f"conv{g % 4}")
                            for i in range(4):
                                nc.tensor.matmul(
                                    tp_t[:, 128 * i:128 * i + 128],
                                    lhsT=exp_ts[i][:],
                                    rhs=ident_t[:],
                                    is_transpose=True,
                                    start=(i == 0), stop=(i == 3),
                                )
                            tpc = softp.tile([128, 512], f32, tag="tpc")
                            nc.scalar.activation(
                                tpc[:], tp_t[:], ACT_F.Copy)
                            ov = out[:].rearrange("n h w c -> n w h c")
                            for i in range(4):
                                bi = 4 * page + i
                                tpv = tpc[:, 128 * i:128 * i + 128] \
                                    .rearrange("p (j q) -> p j q", j=4)
                                expT = softp.tile([128, 96], f32, tag="expT")
                                sums = softp.tile([128, 8], f32, tag="sums")
                                for hoff in range(4):
                                    nc.scalar.activation(
                                        expT[:, 24 * hoff:24 * hoff + 24]
                                        .rearrange("p (j c) -> p j c", j=4),
                                        tpv[:, :, hoff * 6:hoff * 6 + 6],
                                        ACT_F.Copy,
                                        accum_out=sums[:, hoff:hoff + 1])
                                nc.vector.reciprocal(
                                    sums[:, 4:8], sums[:, 0:4])
                                ho_base = 8 * bi + 4 * rho
                                nho = min(4, HO - ho_base)
                                ost = softp.tile([128, 96], f32, tag="ost")
                                for hoff in range(nho):
                                    nc.scalar.activation(
                                        ost[:, 24 * hoff:24 * hoff + 24],
                                        expT[:, 24 * hoff:24 * hoff + 24],
                                        ACT_F.Copy,
                                        scale=sums[:, 4 + hoff:5 + hoff])
                                nc.sync.dma_start(
                                    ov[n, :, ho_base:ho_base + nho],
                                    ost[0:WO, 0:24 * nho].rearrange(
                                        "p (h c) -> p h c", h=nho))

                        pending.append(ep2)
                        if len(pending) > 2:
                            pending.pop(0)()
            for fn in pending:
                fn()
    nc.compile()
    return nc


def _prep_tables(conv_weight, conv_bias):
    Wn = -np.asarray(conv_weight, np.float32)
    # 18 mats of [32 rows, 128 cols]; m = rho*9 + kd*3 + kw
    # rows r = ci*10+h ; cols = 32*j + hoff*6 + cc
    wt = np.zeros((18, 32, 128), np.float32)
    r = np.arange(30)
    ci_r, h_r = r // 10, r % 10
    for m in range(18):
        kw = m % 3
        kd = (m // 3) % 3
        rho = m // 9
        for j in range(4):
            for col in range(24):
                hoff, cc = col // 6, col % 6
                kh = h_r - (4 * rho + hoff)
                msk = (kh >= 0) & (kh < 3)
                wt[m, r[msk], 32 * j + col] = Wn[6 * j + cc, ci_r[msk], kd,
                                                 kh[msk], kw]
    wt_flat = wt.transpose(1, 0, 2).reshape(32, 18 * 128)  # [32, 2304]
    wt128 = np.tile(wt_flat, (4, 1)).astype(_np_mmdt())

    bias = np.zeros((128, 1), np.float32)
    b = np.asarray(conv_bias, np.float32)
    for j in range(4):
        for hoff in range(4):
            for cc in range(6):
                bias[32 * j + hoff * 6 + cc, 0] = b[6 * j + cc]
    return wt128, bias


def _block_x(xc):
    """[NPC,3,32,128,128] -> pre-blocked [4, 128, NPC*34*130] (see program)."""
    blk = np.zeros((4, 128, NPC, 34, 130), _np_mmdt())
    for page in range(4):
        for i in range(4):
            hbase = 8 * (4 * page + i)
            hrows = min(10, H - hbase)
            for ci in range(CI):
                blk[page, 32 * i + 10 * ci:32 * i + 10 * ci + hrows, :, :D, :W] = \
                    xc[:, ci, :, hbase:hbase + hrows, :].transpose(2, 0, 1, 3)
    return blk.reshape(4, 128, NPC * 34 * 130)


def _get_runner():
    """Build the bass program and a cached jitted SPMD executor once."""
    if "runner" in _cache:
        return _cache["runner"]
    import jax
    from jax.experimental.shard_map import shard_map
    from jax.sharding import Mesh, PartitionSpec
    from concourse import bass2jax

    nc = _build_program()
    _cache["nc"] = nc
    bass2jax.install_neuronx_cc_hook()

    import concourse.mybir as mybir

    pname = nc.partition_id_tensor.name if nc.partition_id_tensor else None
    in_names, out_names, out_avals, zero_outs = [], [], [], []
    for alloc in nc.m.functions[0].allocations:
        if not isinstance(alloc, mybir.MemoryLocationSet):
            continue
        name = alloc.memorylocations[0].name
        if alloc.kind == "ExternalInput":
            if name != pname:
                in_names.append(name)
        elif alloc.kind == "ExternalOutput":
            out_names.append(name)
            shape = tuple(alloc.tensor_shape)
            dtype = mybir.dt.np(alloc.dtype)
            out_avals.append(jax.core.ShapedArray(shape, dtype))
            zero_outs.append(np.zeros(shape, dtype))
    n_params = len(in_names)
    n_outs = len(out_avals)
    all_names = in_names + out_names + ([pname] if pname else [])

    def _body(*args):
        operands = list(args)
        if pname:
            operands.append(bass2jax.partition_id_tensor())
        outs = bass2jax._bass_exec_p.bind(
            *operands,
            out_avals=tuple(out_avals),
            in_names=tuple(all_names),
            out_names=tuple(out_names),
            lowering_input_output_aliases=(),
            sim_require_finite=True,
            sim_require_nnan=True,
            nc=nc,
        )
        return tuple(outs)

    devices = jax.devices()[:NCORES]
    mesh = Mesh(np.asarray(devices), ("core",))
    in_specs = (PartitionSpec("core"),) * (n_params + n_outs)
    out_specs = (PartitionSpec("core"),) * n_outs
    donate = tuple(range(n_params, n_params + n_outs))
    sharded = jax.jit(
        shard_map(_body, mesh=mesh, in_specs=in_specs, out_specs=out_specs,
                  check_rep=False),
        donate_argnums=donate, keep_unused=True)

    def run(in_maps):
        per_core = [[np.asarray(m[name]) for name in in_names]
                    for m in in_maps]
        concat_in = [
            np.concatenate([per_core[c][i] for c in range(NCORES)], axis=0)
            for i in range(n_params)
        ]
        concat_zeros = [
            np.zeros((NCORES * z.shape[0], *z.shape[1:]), z.dtype)
            for z in zero_outs
        ]
        out_arrs = sharded(*concat_in, *concat_zeros)
        return [
            {name: np.asarray(out_arrs[i]).reshape(
                NCORES, *out_avals[i].shape)[c]
             for i, name in enumerate(out_names)}
            for c in range(NCORES)
        ]

    _cache["runner"] = run
    return run


def kernel(x, conv_weight, conv_bias):
    x = np.asarray(x, np.float32)
    wt128, bias = _prep_tables(conv_weight, conv_bias)
    ident = np.eye(128, dtype=np.float32)

    run = _get_runner()
    in_maps = [
        {
            "xs": _block_x(x[NPC * c:NPC * (c + 1)]),
            "wt": wt128,
            "bias": bias,
            "ident": ident,
        }
        for c in range(NCORES)
    ]
    results = run(in_maps)
    outs = [results[c]["out"] for c in range(NCORES)]
    full = np.concatenate(outs, axis=0)  # [16,126,126,24] (n,ho,wo,co)
    full = np.ascontiguousarray(full.transpose(0, 3, 1, 2))
    return full.reshape(N_TOT, CO, 1, HO, WO).astype(np.float32)


# revision 28
# speedup vs baseline: 1.0112x; 1.0112x over previous
"""Trainium2 Bass kernel for: Conv3d(3,24,k=3,VALID) -> min over depth -> softmax over channels.

Input  x: [16,3,32,128,128] f32, conv_weight [24,3,3,3,3], conv_bias [24].
Output: [16,24,1,126,126] f32.

Strategy (per core; batch-sharded 2 samples/core over 8 cores):
 - x in SBUF as 4 "pages" of 4 h-blocks; each 32-partition block holds
   rows r = ci*10 + h_local (10 h-rows x 3 channels, 2 pad rows) with the
   free dim = (n, d, w).
 - Conv via 16-way 32x32 TensorE tiling: tile (i,j) = (h-block i, co-chunk j),
   M = 24 = (hoff4 x co6) per ho-quad rho, K covers (kh,ci) via Toeplitz
   weights, (kd,kw) = 9 PSUM-accumulation passes with shifted rhs APs.
   dtype float32r (fp32 data on the PE fast path; ~1e-3 on HW).
 - Weights NEGATED on host so min-over-depth becomes a free-dim reduce max.
 - DVE tensor_reduce(max) folds depth 30->1 straight out of PSUM.
 - ACT computes exp(bias - m) (= exp(min(y)+bias)); PE transposes exp tiles;
   ACT Copy+accum_out builds softmax denominators; ACT Copy+scale divides.
"""
import sys

sys.path.insert(0, "/opt/trn_rl_repo")

import numpy as np

# Problem constants
N_TOT, CI, D, H, W = 16, 3, 32, 128, 128
CO = 24
DO, HO, WO = 30, 126, 126
NCORES = 8
NPC = N_TOT // NCORES  # samples per core = 2

# dtype for matmul operands (PSUM accumulation is always fp32)
MM_DT = "float32r"

_cache = {}


def _np_mmdt():
    if MM_DT == "float32" or MM_DT == "float32r":
        return np.float32
    if MM_DT == "float16":
        return np.float16
    import ml_dtypes
    return ml_dtypes.bfloat16


def _build_program():
    import concourse.bass as bass
    import concourse.mybir as mybir
    from concourse import bacc, tile

    dt = mybir.dt
    mdt = getattr(dt, MM_DT)
    f32 = dt.float32
    AX = mybir.AxisListType
    ALU = mybir.AluOpType
    ACT_F = mybir.ActivationFunctionType

    nc = bacc.Bacc("TRN2", target_bir_lowering=False, debug=False)

    # host-pre-blocked x: [page, partition, n*34*130] with partition =
    # 32*i + 10*ci + h_local, (d,w) zero-padded to (34,130) for bank-fill
    xs = nc.dram_tensor("xs", [4, 128, NPC * 34 * 130], mdt,
                        kind="ExternalInput")
    wt = nc.dram_tensor("wt", [128, 72 * 32], mdt, kind="ExternalInput")
    bias = nc.dram_tensor("bias", [128, 1], f32, kind="ExternalInput")
    ident = nc.dram_tensor("ident", [128, 128], f32, kind="ExternalInput")
    # [n, ho, wo, co] so output DMA descriptors are 96B-contiguous runs
    out = nc.dram_tensor("out", [NPC, HO, WO, CO], f32, kind="ExternalOutput")

    with tile.TileContext(nc) as tc:
        with (
            tc.tile_pool(name="const", bufs=1) as constp,
            tc.tile_pool(name="xpage", bufs=2) as xpagep,
            tc.tile_pool(name="expp", bufs=6) as expp,
            tc.tile_pool(name="soft", bufs=4) as softp,
            tc.tile_pool(name="ps", bufs=2, space="PSUM") as psp,
        ):
            wt_t = constp.tile([128, 72 * 32], mdt)
            nc.sync.dma_start(wt_t[:], wt[:])
            bias_t = constp.tile([128, 1], f32)
            nc.sync.dma_start(bias_t[:], bias[:])
            ident_t = constp.tile([128, 128], f32)
            nc.sync.dma_start(ident_t[:], ident[:])

            DP, WP = 34, 130  # padded (d, w) extents per partition
            FREE = NPC * DP * WP  # free elems per partition of a page

            pending = []  # queue of epilogue-2 closures (lagged)

            for page in range(4):
                page_t = xpagep.tile([128, FREE], mdt)
                pv = page_t[:].rearrange("p (n d w) -> p n d w", n=NPC, d=DP, w=WP)
                nc.sync.dma_start(page_t[:], xs[page])

                for rho in range(2):
                    for n in range(NPC):
                        exp_ts = [expp.tile([128, 128], f32, tag=f"exp{i}",
                                            name=f"exp_{i}")
                                  for i in range(4)]
                        for wc in range(8):
                            convs, cvs = [], []
                            for i in range(4):
                                conv_t = psp.tile([128, 512], f32,
                                                  tag=f"conv{i}")
                                convs.append(conv_t)
                                cvs.append(conv_t[:, 0:480].rearrange(
                                    "p (w d) -> p w d", d=30))
                            # i innermost: the 4 row-tiles stream
                            # concurrently in the 32x128-tiled PE array
                            for p9 in range(9):
                                kd, kw = p9 // 3, p9 % 3
                                m = (rho * 9) + kd * 3 + kw
                                for i in range(4):
                                    lhsT = wt_t[32 * i:32 * i + 32,
                                                128 * m:128 * m + 128]
                                    rhs = pv[32 * i:32 * i + 32, n,
                                             kd:kd + 30,
                                             wc * 16 + kw:wc * 16 + kw + 16
                                             ].rearrange("p d w -> p w d")
                                    nc.tensor.matmul(
                                        cvs[i],
                                        lhsT=lhsT,
                                        rhs=rhs,
                                        start=(p9 == 0),
                                        stop=False,
                                        tile_position=(32 * i, 0),
                                    )
                            for i in range(4):
                                # bank-fill: 32 junk cols so the PSUM zero
                                # region is fully written before the read
                                nc.tensor.matmul(
                                    convs[i][:, 480:512],
                                    lhsT=wt_t[32 * i:32 * i + 32, 0:128],
                                    rhs=pv[32 * i:32 * i + 32, n, 0, 0:32],
                                    start=False, stop=True,
                                    tile_position=(32 * i, 0),
                                )
                            for i in range(4):
                                # depth-max out of PSUM
                                nc.vector.tensor_reduce(
                                    exp_ts[i][:, wc * 16:wc * 16 + 16],
                                    cvs[i], axis=AX.X, op=ALU.max)
                        for i in range(4):
                            # exp(bias - m) in-place on the reduced tile
                            nc.scalar.activation(
                                exp_ts[i][:], exp_ts[i][:], ACT_F.Exp,
                                bias=bias_t[:, 0:1], scale=-1.0)

                        def ep2(page=page, rho=rho, n=n, exp_ts=exp_ts):
                            tp_t = psp.tile([128, 512], f32, tag="conv0")
                            for i in range(4):
                                nc.tensor.matmul(
                                    tp_t[:, 128 * i:128 * i + 128],
                                    lhsT=exp_ts[i][:],
                                    rhs=ident_t[:],
                                    is_transpose=True,
                                    start=(i == 0), stop=(i == 3),
                                )
                            tpc = softp.tile([128, 512], f32, tag="tpc")
                            nc.scalar.activation(
                                tpc[:], tp_t[:], ACT_F.Copy)
                            ov = out[:].rearrange("n h w c -> n w h c")
                            for i in range(4):
                                bi = 4 * page + i
                                tpv = tpc[:, 128 * i:128 * i + 128] \
                                    .rearrange("p (j q) -> p j q", j=4)
                                expT = softp.tile([128, 96], f32, tag="expT")
                                sums = softp.tile([128, 8], f32, tag="sums")
                                for hoff in range(4):
                                    nc.scalar.activation(
                                        expT[:, 24 * hoff:24 * hoff + 24]
                                        .rearrange("p (j c) -> p j c", j=4),
                                        tpv[:, :, hoff * 6:hoff * 6 + 6],
                                        ACT_F.Copy,
                                        accum_out=sums[:, hoff:hoff + 1])
                                nc.vector.reciprocal(
                                    sums[:, 4:8], sums[:, 0:4])
                                ho_base = 8 * bi + 4 * rho
                                nho = min(4, HO - ho_base)
                                ost = softp.tile([128, 96], f32, tag="ost")
                                for hoff in range(nho):
                                    nc.scalar.activation(
                                        ost[:, 24 * hoff:24 * hoff + 24],
                                        expT[:, 24 * hoff:24 * hoff + 24],
                                        ACT_F.Copy,
                                        scale=sums[:, 4 + hoff:5 + hoff])
                                nc.sync.dma_start(
                                    ov[n, :, ho_base:ho_base + nho],
                                    ost[0:WO, 0:24 * nho].rearrange(
                                        "p (h c) -> p h c", h=nho))

                        pending.append(ep2)
                        if len(pending) > 1:
                            pending.pop(0)()
            for fn in pending:
                fn()
    nc.compile()
    return nc


def _prep_tables(conv_weight, conv_bias):
    Wn = -np.asarray(conv_weight, np.float32)
    # 18 mats of [32 rows, 128 cols]; m = rho*9 + kd*3 + kw
    # rows r = ci*10+h ; cols = 32*j + hoff*6 + cc
    wt = np.zeros((18, 32, 128), np.float32)
    r = np.arange(30)
    ci_r, h_r = r // 10, r % 10
    for m in range(18):
        kw = m % 3
        kd = (m // 3) % 3
        rho = m // 9
        for j in range(4):
            for col in range(24):
                hoff, cc = col // 6, col % 6
                kh = h_r - (4 * rho + hoff)
                msk = (kh >= 0) & (kh < 3)
                wt[m, r[msk], 32 * j + col] = Wn[6 * j + cc, ci_r[msk], kd,
                                                 kh[msk], kw]
    wt_flat = wt.transpose(1, 0, 2).reshape(32, 18 * 128)  # [32, 2304]
    wt128 = np.tile(wt_flat, (4, 1)).astype(_np_mmdt())

    bias = np.zeros((128, 1), np.float32)
    b = np.asarray(conv_bias, np.float32)
    for j in range(4):
        for hoff in range(4):
            for cc in range(6):
                bias[32 * j + hoff * 6 + cc, 0] = b[6 * j + cc]
    return wt128, bias


def _block_x(xc):
    """[NPC,3,32,128,128] -> pre-blocked [4, 128, NPC*34*130] (see program)."""
    blk = np.zeros((4, 128, NPC, 34, 130), _np_mmdt())
    for page in range(4):
        for i in range(4):
            hbase = 8 * (4 * page + i)
            hrows = min(10, H - hbase)
            for ci in range(CI):
                blk[page, 32 * i + 10 * ci:32 * i + 10 * ci + hrows, :, :D, :W] = \
                    xc[:, ci, :, hbase:hbase + hrows, :].transpose(2, 0, 1, 3)
    return blk.reshape(4, 128, NPC * 34 * 130)


def _get_runner():
    """Build the bass program and a cached jitted SPMD executor once."""
    if "runner" in _cache:
        return _cache["runner"]
    import jax
    from jax.experimental.shard_map import shard_map
    from jax.sharding import Mesh, PartitionSpec
    from concourse import bass2jax

    nc = _build_program()
    _cache["nc"] = nc
    bass2jax.install_neuronx_cc_hook()

    import concourse.mybir as mybir

    pname = nc.partition_id_tensor.name if nc.partition_id_tensor else None
    in_names, out_names, out_avals, zero_outs = [], [], [], []
    for alloc in nc.m.functions[0].allocations:
        if not isinstance(alloc, mybir.MemoryLocationSet):
            continue
        name = alloc.memorylocations[0].name
        if alloc.kind == "ExternalInput":
            if name != pname:
                in_names.append(name)
        elif alloc.kind == "ExternalOutput":
            out_names.append(name)
            shape = tuple(alloc.tensor_shape)
            dtype = mybir.dt.np(alloc.dtype)
            out_avals.append(jax.core.ShapedArray(shape, dtype))
            zero_outs.append(np.zeros(shape, dtype))
    n_params = len(in_names)
    n_outs = len(out_avals)
    all_names = in_names + out_names + ([pname] if pname else [])

    def _body(*args):
        operands = list(args)
        if pname:
            operands.append(bass2jax.partition_id_tensor())
        outs = bass2jax._bass_exec_p.bind(
            *operands,
            out_avals=tuple(out_avals),
            in_names=tuple(all_names),
            out_names=tuple(out_names),
            lowering_input_output_aliases=(),
            sim_require_finite=True,
            sim_require_nnan=True,
            nc=nc,
        )
        return tuple(outs)

    devices = jax.devices()[:NCORES]
    mesh = Mesh(np.asarray(devices), ("core",))
    in_specs = (PartitionSpec("core"),) * (n_params + n_outs)
    out_specs = (PartitionSpec("core"),) * n_outs
    donate = tuple(range(n_params, n_params + n_outs))
    sharded = jax.jit(
        shard_map(_body, mesh=mesh, in_specs=in_specs, out_specs=out_specs,
                  check_rep=False),
        donate_argnums=donate, keep_unused=True)

    def run(in_maps):
        per_core = [[np.asarray(m[name]) for name in in_names]
                    for m in in_maps]
        concat_in = [
            np.concatenate([per_core[c][i] for c in range(NCORES)], axis=0)
            for i in range(n_params)
        ]
        concat_zeros = [
            np.zeros((NCORES * z.shape[0], *z.shape[1:]), z.dtype)
            for z in zero_outs
        ]
        out_arrs = sharded(*concat_in, *concat_zeros)
        return [
            {name: np.asarray(out_arrs[i]).reshape(
                NCORES, *out_avals[i].shape)[c]
             for i, name in enumerate(out_names)}
            for c in range(NCORES)
        ]

    _cache["runner"] = run
    return run


def kernel(x, conv_weight, conv_bias):
    x = np.asarray(x, np.float32)
    wt128, bias = _prep_tables(conv_weight, conv_bias)
    ident = np.eye(128, dtype=np.float32)

    run = _get_runner()
    in_maps = [
        {
            "xs": _block_x(x[NPC * c:NPC * (c + 1)]),
            "wt": wt128,
            "bias": bias,
            "ident": ident,
        }
        for c in range(NCORES)
    ]
    results = run(in_maps)
    outs = [results[c]["out"] for c in range(NCORES)]
    full = np.concatenate(outs, axis=0)  # [16,126,126,24] (n,ho,wo,co)
    full = np.ascontiguousarray(full.transpose(0, 3, 1, 2))
    return full.reshape(N_TOT, CO, 1, HO, WO).astype(np.float32)
